# revision 43
# baseline (speedup 1.0000x reference)
"""GNN message-passing kernel for Trainium2 (8 NeuronCores, batch-sharded).

Computes, for each batch b:
    neigh[i, d] = max(0, max_{j: A[b,j,i]=1} x[b, j, d])
    out = x @ W_self.T + neigh @ W_neigh.T

Algorithm: log-sum-exp relaxation of the masked max with the relu folded
into the sum via a ones-row:
    masked_relu_max[i, d] ~= (1/t) * ln( 1 + sum_j A[j, i] * exp(t * x[j, d]) )
with t = 16 (t*|x| < 83 so exp stays in f32 range; the +1 comes from one
extra contraction row with E=1, A=1). The Ln input is prescaled by 2^-64
(exact) to stay inside the scalar engine's [-2^64, 2^64] domain. The
resulting -64*ln2 shift of the Ln output passes linearly through the
neighbor matmul as a constant per-output-row offset
    bias[e] = 64*ln2/t * sum_d W_neigh[e, d]
which the HOST adds after readback (f32-exact, zero device cost). 1/t is
folded into W_neigh on the host. There is no relu/shift stage on device:
Ln writes bf16 nT directly and the PE consumes it.

Everything is computed transposed so no PE transposes are needed:
M'^T[d,i] = sum_j E[j,d]*A[j,i] takes E and A in natural j-major layout;
the finals out^T[e,s] = W_self^T(lhsT) @ x^T + (W_neigh^T/t) @ nT take the
host-supplied x^T and the Ln result nT as streaming rhs.

Host-side packing per core (BPC=4 batches b0..b3, J0=128 "main" j rows;
22 tail rows + 1 ones-row = 23 rows per batch; matmul base partitions must
be 0/32/64 so at most 3 tail blocks share a 128-col block). A-tails ride
inside the x tensors (small, and they close every M accumulation group so
they must land early):
    xb0 [128, 534] bf16  x mains b0,b1 (0:256) | x tails b0@r0, b1@r32,
                         b2@r64 (256:384, exp'd; ones-row = 0) | A tails
                         b0@r0, b1@r32, b2@r64 (384:534, raw; ones-row = 1)
    xb1 [128, 534] bf16  x mains b2,b3 | x tail b3@r0 | A tail b3@r0
    am0 [128, 300] bf16  A mains b0,b1
    am1 [128, 300] bf16  A mains b2,b3
    wxa [128, 556] bf16  [W_self.T | W_neigh.T/t | x^T b0,b1]
    xtb [128, 300] bf16  x^T b2,b3
    op  [128, 600] bf16  out^T (e-major, batch blocks of 150); host
                         upcasts, adds bias[e], transposes back.

Queue plan (HWDGE slots are FIFO: SP#0, Act#0, SP#1, SP#2; Pool-SWDGE runs
in parallel): SP: xb0, am1, xtb; Act: am0; Pool: xb1, wxa. Outputs leave
as two [128,300] DMAs: {b0,b1} via Act, {b2,b3} via SP.

Cost-model-driven scheduling tricks (each verified against TimelineSim):
  - one combined exp+ln activation table loaded explicitly up front (the
    tile scheduler otherwise budgets a mid-kernel table switch and inverts
    the PE instruction order);
  - PE pstate warmup: dummy K=1 matmuls keep the tensor engine busy from
    ~1.2us so the real matmuls run at the full-rate p-state;
  - self matmuls open each output PSUM group (neigh closes it) and their
    rhs is "poked" by a value-preserving DVE op reading exp output, which
    dependency-orders them after the M phase on the in-order PE stream;
  - framework const-tile memsets that nothing reads are dropped and the
    surviving one is moved off the Pool preamble path.
"""

import numpy as np
import ml_dtypes

import concourse.bacc as bacc
import concourse.bass as bass
import concourse.mybir as mybir
import concourse.tile as tile
from concourse.bass_utils import run_bass_kernel_spmd

B, S, D = 32, 150, 128
NCORES = 8
BPC = B // NCORES  # batches per core
J0 = 128  # full-partition j rows
JT = S - J0  # 22 real tail rows
TR = JT + 1  # tail rows incl the ones-row
T_LSE = 16.0
LN2_64 = float(64 * np.log(2.0))

f32 = mybir.dt.float32
bf16 = mybir.dt.bfloat16

# tail block home: batch -> (tile index, base partition row)
TAIL_HOME = {0: (0, 0), 1: (0, 32), 2: (0, 64), 3: (1, 0)}

_PROGRAM_CACHE: dict[str, bass.Bass] = {}


def _merge_act_table_loads(nc):
    """One table serves exp and ln; retarget the first greedy load and drop
    the rest (a mid-kernel table switch costs 1283 ns on the Act engine)."""
    from concourse.hw_specs import get_activation_tables

    tabs = list(get_activation_tables(nc.m.arch).items())
    target = next(
        i
        for i, (_, funcs) in enumerate(tabs)
        if mybir.ActivationFunctionType.Exp in funcs
        and mybir.ActivationFunctionType.Ln in funcs
    )
    for blk in nc.main_func.blocks:
        loads = [
            ins
            for ins in blk.instructions
            if isinstance(ins, mybir.InstLoadActFuncSet)
        ]
        if not loads:
            continue
        loads[0].act_func_set_id = target
        for ins in loads[1:]:
            blk.instructions.remove(ins)


def _drop_dead_const_memsets(nc):
    """The framework materializes several [128,1] constant tiles (1.0 in two
    dtypes, uint8-127) with Pool memsets that serialize ahead of the entry
    barrier; only const-float32-0.0 (the activations' bias operand) is ever
    read here. Drop the unread ones -- each costs ~95 ns of pre-barrier
    Pool time."""
    read = set()
    for blk in nc.main_func.blocks:
        for ins in blk.instructions:
            try:
                for ap in list(ins.ins):
                    s = str(ap)
                    if "const-" in s:
                        read.add(s.split("memref='")[1].split("'")[0])
            except Exception:
                pass
    for blk in nc.main_func.blocks:
        dead = [
            ins
            for ins in blk.instructions
            if isinstance(ins, mybir.InstMemset)
            and "const-" in str(ins.outs[0])
            and str(ins.outs[0]).split("memref='")[1].split("'")[0] not in read
        ]
        for ins in dead:
            blk.instructions.remove(ins)
        # the surviving const memset (exp/ln bias tile) runs on DVE so the
        # preamble barrier isn't serialized behind Pool
        for ins in blk.instructions:
            if isinstance(ins, mybir.InstMemset) and "const-" in str(ins.outs[0]):
                ins.engine = mybir.EngineType.DVE


def _build_program() -> bass.Bass:
    if "nc" in _PROGRAM_CACHE:
        return _PROGRAM_CACHE["nc"]

    nc = bacc.Bacc("TRN2", target_bir_lowering=False, debug=False)
    xb0a_d = nc.dram_tensor("xb0a", [128, 384], bf16, kind="ExternalInput").ap()
    xb0b_d = nc.dram_tensor("xb0b", [128, 150], bf16, kind="ExternalInput").ap()
    xb1_d = nc.dram_tensor("xb1", [128, 534], bf16, kind="ExternalInput").ap()
    am0_d = nc.dram_tensor("am0", [128, 300], bf16, kind="ExternalInput").ap()
    am1_d = nc.dram_tensor("am1", [128, 300], bf16, kind="ExternalInput").ap()
    wxa_d = nc.dram_tensor("wxa", [128, 556], bf16, kind="ExternalInput").ap()
    xtb_d = nc.dram_tensor("xtb", [128, 300], bf16, kind="ExternalInput").ap()
    op_d = nc.dram_tensor("op", [128, BPC * S], bf16, kind="ExternalOutput").ap()

    with tile.TileContext(nc) as tc:
        with (
            tc.tile_pool(name="const", bufs=1) as cpool,
            tc.tile_pool(name="work", bufs=1) as wpool,
            tc.tile_pool(name="psum", bufs=1, space="PSUM") as ppool,
        ):
            xb0 = wpool.tile([128, 534], bf16, tag="xb0", name="xb0")
            xb1 = wpool.tile([128, 534], bf16, tag="xb1", name="xb1")
            xb = [xb0, xb1]
            am = [
                wpool.tile([128, 300], bf16, tag=f"am{h}", name=f"am{h}")
                for h in range(2)
            ]
            wxa = cpool.tile([128, 556], bf16, tag="wxa")
            xtb = cpool.tile([128, 300], bf16, tag="xtb")

            # Input DMAs. Bus/sem landing order by design: xb0, xb1, am0,
            # am1, wxa, xtb -- matching the order consumers need them.
            nc.sync.dma_start(xb0[:, 0:384], xb0a_d[:, :])
            nc.gpsimd.dma_start(xb[1][:], xb1_d[:, :])
            nc.scalar.dma_start(xb0[:, 384:534], xb0b_d[:, :])
            nc.sync.dma_start(am[0][:], am0_d[:, :])
            nc.sync.dma_start(am[1][:], am1_d[:, :])
            nc.gpsimd.dma_start(wxa[:], wxa_d[:, :])
            nc.sync.dma_start(xtb[:], xtb_d[:, :])

            wst = wxa[:, 0:D]
            wnt = wxa[:, D : 2 * D]

            # Load the one activation table that serves BOTH exp and ln up
            # front. Without this, the greedy insertion pass (and, worse,
            # the tile scheduler's internal cost model) charges a 1283 ns
            # table load before the first exp AND before the first Ln --
            # the latter skews the scheduler into thinking exp finishes
            # late, which made it front-load the wxa/xtb-gated self matmuls
            # ahead of the M matmuls on the in-order PE stream.
            from concourse.hw_specs import get_activation_tables

            _tabs = list(get_activation_tables(nc.m.arch).items())
            _combined = next(
                i
                for i, (_, funcs) in enumerate(_tabs)
                if mybir.ActivationFunctionType.Exp in funcs
                and mybir.ActivationFunctionType.Ln in funcs
            )
            _atl = mybir.InstLoadActFuncSet(
                name=nc.get_next_instruction_name(),
                ins=[],
                outs=[],
                act_func_set_id=_combined,
            )
            nc.scalar.add_instruction(_atl)

            # E = exp(t*x), one op per half (mains + resident x-tails;
            # the raw A-tail cols 384:534 are not exp'd)
            ec = [
                wpool.tile([128, 384], bf16, tag=f"ec{h}", name=f"ec{h}")
                for h in range(2)
            ]
            for h in range(2):
                nc.scalar.activation(
                    ec[h][:],
                    xb[h][:, 0:384],
                    mybir.ActivationFunctionType.Exp,
                    scale=T_LSE,
                )

            # PE pstate warmup: the tensor engine reaches full speed only
            # after ~3us of continuous execution. Keep it busy from the
            # start with dummy K=1 matmuls over a memset scratch row so the
            # real matmuls issue into a warm (full-rate) engine.
            wsrc = wpool.tile([1, 520], bf16, tag="wsrc")
            wps = ppool.tile([1, 512], f32, tag="wps")
            nc.vector.memset(wsrc[:], 0.0)
            for w in range(5):
                nc.tensor.matmul(
                    wps[:], wsrc[0:1, 0:1], wsrc[0:1, 8:520], start=True, stop=True
                )

            mM = [ppool.tile([128, 2 * S], f32, tag=f"mM{p}", name=f"mM{p}") for p in range(2)]
            mO = [
                ppool.tile([128, S], f32, tag=f"mO{b}", name=f"mO{b}")[:]
                for b in range(BPC)
            ]

            # M'^T = sum_j E[j,d] * A[j,i] (+ ones-row), per batch; all M
            # matmuls emitted before any O work so PE never stalls on Ln.
            for b in range(BPC):
                p, q = divmod(b, 2)
                th, tr = TAIL_HOME[b]
                nc.tensor.matmul(
                    mM[p][:, q * S : (q + 1) * S],
                    ec[p][:, q * D : (q + 1) * D],
                    am[p][:, q * S : (q + 1) * S],
                    start=True,
                    stop=False,
                )
                nc.tensor.matmul(
                    mM[p][:, q * S : (q + 1) * S],
                    ec[th][tr : tr + TR, 256:384],
                    xb[th][tr : tr + TR, 384:534],
                    start=False,
                    stop=True,
                )

            nT = wpool.tile([128, BPC * S], bf16, tag="nT")
            osb = wpool.tile([128, BPC * S], bf16, tag="osb")

            # nT = ln(2^-64 * M') in bf16, straight from PSUM, per pair
            for p in range(2):
                nc.scalar.activation(
                    nT[:, p * 2 * S : (p + 1) * 2 * S],
                    mM[p][:],
                    mybir.ActivationFunctionType.Ln,
                    scale=2.0**-64,
                )

            # Output matmuls per batch: self terms first (ready early via
            # wxt), neighbor terms interleaved to close each batch's group
            # as soon as its nT half lands.
            # The self matmul OPENS each batch's PSUM group and the cheap
            # neighbor term closes it, so the output copies gate on the
            # neighbor matmul alone. To stop the scheduler hoisting the
            # self matmuls ahead of the M matmuls (in-order PE stream ->
            # head-of-line blocking), one element of each self rhs region is
            # "poked" in place by a value-preserving DVE op (0*ec1 + self)
            # that REALLY reads exp pair 1's output, so the selfs are
            # dependency-ordered after the exp chain.
            def _poke(ap_elem):
                nc.vector.scalar_tensor_tensor(
                    out=ap_elem,
                    in0=ec[1][0:1, 0:1],
                    scalar=0.0,
                    in1=ap_elem,
                    op0=mybir.AluOpType.mult,
                    op1=mybir.AluOpType.add,
                )

            _poke(wxa[0:1, 256:257])
            _poke(wxa[0:1, 256 + S : 257 + S])
            _poke(xtb[0:1, 0:1])
            _poke(xtb[0:1, S : S + 1])

            def self_mm(b):
                rhs = (
                    wxa[:, 256 + b * S : 256 + (b + 1) * S]
                    if b < 2
                    else xtb[:, (b - 2) * S : (b - 1) * S]
                )
                nc.tensor.matmul(mO[b], wst, rhs, start=True, stop=False)

            def neigh_mm(b):
                nc.tensor.matmul(
                    mO[b],
                    wnt,
                    nT[:, b * S : (b + 1) * S],
                    start=False,
                    stop=True,
                )

            for b in range(BPC):
                self_mm(b)
            for b in range(BPC):
                neigh_mm(b)

            # PSUM -> SBUF staging per batch, alternating DVE / Act
            for b in range(BPC):
                dst = osb[:, b * S : (b + 1) * S]
                if b in (0, 3):
                    nc.scalar.copy(dst, mO[b])
                else:
                    nc.vector.tensor_copy(out=dst, in_=mO[b])

            # two output DMAs; outA is emitted after every copy so the Act
            # SEQ (queue depth 0) never blocks a pending copy on its wait.
            nc.scalar.dma_start(op_d[:, 0 : 2 * S], osb[:, 0 : 2 * S])
            nc.sync.dma_start(op_d[:, 2 * S : 4 * S], osb[:, 2 * S : 4 * S])

    nc.compile()
    _merge_act_table_loads(nc)
    _drop_dead_const_memsets(nc)
    _PROGRAM_CACHE["nc"] = nc
    return nc


def pack_inputs(x, A, W_self, W_neigh):
    """Per-core input dicts; all packing/casting on host."""
    x = np.asarray(x, dtype=np.float32)
    A = np.asarray(A)
    wst = np.ascontiguousarray(np.asarray(W_self, dtype=np.float32).T).astype(
        ml_dtypes.bfloat16
    )
    wnt = np.ascontiguousarray(
        np.asarray(W_neigh, dtype=np.float32).T / np.float32(T_LSE)
    ).astype(ml_dtypes.bfloat16)

    maps = []
    for c in range(NCORES):
        xs = x[c * BPC : (c + 1) * BPC]  # [BPC, S, D]
        As = A[c * BPC : (c + 1) * BPC]  # [BPC, S, S]
        xba = [np.zeros((128, 534), dtype=ml_dtypes.bfloat16) for _ in range(2)]
        ama = [np.zeros((128, 300), dtype=ml_dtypes.bfloat16) for _ in range(2)]
        for b in range(BPC):
            p, q = divmod(b, 2)
            th, tr = TAIL_HOME[b]
            xcast = xs[b].astype(ml_dtypes.bfloat16)
            acast = As[b].astype(ml_dtypes.bfloat16)
            xba[p][:, q * D : (q + 1) * D] = xcast[:J0, :]
            xba[th][tr : tr + JT, 256:384] = xcast[J0:, :]
            xba[th][tr : tr + JT, 384:534] = acast[J0:, :]
            xba[th][tr + JT, 384:534] = np.float32(1.0)
            ama[p][:, q * S : (q + 1) * S] = acast[:J0, :]
        xT = (
            np.ascontiguousarray(xs.transpose(2, 0, 1))
            .reshape(D, BPC * S)
            .astype(ml_dtypes.bfloat16)
        )
        wxa = np.ascontiguousarray(
            np.concatenate([wst, wnt, xT[:, 0 : 2 * S]], axis=1)
        )
        xtb = np.ascontiguousarray(xT[:, 2 * S : 4 * S])
        maps.append(
            {
                "xb0a": np.ascontiguousarray(xba[0][:, 0:384]),
                "xb0b": np.ascontiguousarray(xba[0][:, 384:534]),
                "xb1": xba[1],
                "am0": ama[0],
                "am1": ama[1],
                "wxa": wxa,
                "xtb": xtb,
            }
        )
    return maps


def _bias(W_neigh):
    """Host-side correction for the -64*ln2 Ln shift through W_neigh."""
    return np.float32(LN2_64) * (
        np.asarray(W_neigh, dtype=np.float32) / np.float32(T_LSE)
    ).sum(axis=1)


def unpack_output(res_out, bias):
    """op [D, BPC*S] (= out^T, e-major) + bias[e] -> [BPC, S, D]"""
    o = np.asarray(res_out, dtype=np.float32) + bias[:, None]
    return np.ascontiguousarray(o.reshape(D, BPC, S).transpose(1, 2, 0))


def kernel(x, A, W_self, W_neigh, **kwargs):
    nc = _build_program()
    in_maps = pack_inputs(x, A, W_self, W_neigh)
    bias = _bias(W_neigh)
    res = run_bass_kernel_spmd(nc, in_maps, core_ids=list(range(NCORES)), **kwargs)
    out = np.concatenate(
        [unpack_output(res.results[c]["op"], bias) for c in range(NCORES)], axis=0
    )
    return np.ascontiguousarray(out.astype(np.float32))


# revision 44
# speedup vs baseline: 1.0680x; 1.0680x over previous
"""GNN message-passing kernel for Trainium2 (8 NeuronCores, batch-sharded).

Computes, for each batch b:
    neigh[i, d] = max(0, max_{j: A[b,j,i]=1} x[b, j, d])
    out = x @ W_self.T + neigh @ W_neigh.T

Algorithm: log-sum-exp relaxation of the masked max with the relu folded
into the sum via a ones-row:
    masked_relu_max[i, d] ~= (1/t) * ln( 1 + sum_j A[j, i] * exp(t * x[j, d]) )
with t = 16 (t*|x| < 83 so exp stays in f32 range; the +1 comes from one
extra contraction row with E=1, A=1). The Ln input is prescaled by 2^-64
(exact) to stay inside the scalar engine's [-2^64, 2^64] domain. The
resulting -64*ln2 shift of the Ln output passes linearly through the
neighbor matmul as a constant per-output-row offset
    bias[e] = 64*ln2/t * sum_d W_neigh[e, d]
which the HOST adds after readback (f32-exact, zero device cost). 1/t is
folded into W_neigh on the host. There is no relu/shift stage on device:
Ln writes bf16 nT directly and the PE consumes it.

Everything is computed transposed so no PE transposes are needed:
M'^T[d,i] = sum_j E[j,d]*A[j,i] takes E and A in natural j-major layout;
the finals out^T[e,s] = W_self^T(lhsT) @ x^T + (W_neigh^T/t) @ nT take the
host-supplied x^T and the Ln result nT as streaming rhs.

Host-side packing per core (BPC=4 batches b0..b3, J0=128 "main" j rows;
22 tail rows + 1 ones-row = 23 rows per batch; matmul base partitions must
be 0/32/64 so at most 3 tail blocks share a 128-col block). A-tails ride
inside the x tensors (small, and they close every M accumulation group so
they must land early):
    xb0 [128, 534] bf16  x mains b0,b1 (0:256) | x tails b0@r0, b1@r32,
                         b2@r64 (256:384, exp'd; ones-row = 0) | A tails
                         b0@r0, b1@r32, b2@r64 (384:534, raw; ones-row = 1)
    xb1 [128, 534] bf16  x mains b2,b3 | x tail b3@r0 | A tail b3@r0
    am0 [128, 300] bf16  A mains b0,b1
    am1 [128, 300] bf16  A mains b2,b3
    wxa [128, 556] bf16  [W_self.T | W_neigh.T/t | x^T b0,b1]
    xtb [128, 300] bf16  x^T b2,b3
    op  [128, 600] bf16  out^T (e-major, batch blocks of 150); host
                         upcasts, adds bias[e], transposes back.

Queue plan (HWDGE slots are FIFO: SP#0, Act#0, SP#1, SP#2; Pool-SWDGE runs
in parallel): SP: xb0, am1, xtb; Act: am0; Pool: xb1, wxa. Outputs leave
as two [128,300] DMAs: {b0,b1} via Act, {b2,b3} via SP.

Cost-model-driven scheduling tricks (each verified against TimelineSim):
  - one combined exp+ln activation table loaded explicitly up front (the
    tile scheduler otherwise budgets a mid-kernel table switch and inverts
    the PE instruction order);
  - PE pstate warmup: dummy K=1 matmuls keep the tensor engine busy from
    ~1.2us so the real matmuls run at the full-rate p-state;
  - self matmuls open each output PSUM group (neigh closes it) and their
    rhs is "poked" by a value-preserving DVE op reading exp output, which
    dependency-orders them after the M phase on the in-order PE stream;
  - framework const-tile memsets that nothing reads are dropped and the
    surviving one is moved off the Pool preamble path.
"""

import numpy as np
import ml_dtypes

import concourse.bacc as bacc
import concourse.bass as bass
import concourse.mybir as mybir
import concourse.tile as tile
from concourse.bass_utils import run_bass_kernel_spmd

B, S, D = 32, 150, 128
NCORES = 8
BPC = B // NCORES  # batches per core
J0 = 128  # full-partition j rows
JT = S - J0  # 22 real tail rows
TR = JT + 1  # tail rows incl the ones-row
T_LSE = 16.0
LN2_64 = float(64 * np.log(2.0))

f32 = mybir.dt.float32
bf16 = mybir.dt.bfloat16

# tail block home: batch -> (tile index, base partition row)
TAIL_HOME = {0: (0, 0), 1: (0, 32), 2: (0, 64), 3: (1, 0)}

_PROGRAM_CACHE: dict[str, bass.Bass] = {}


def _merge_act_table_loads(nc):
    """One table serves exp and ln; retarget the first greedy load and drop
    the rest (a mid-kernel table switch costs 1283 ns on the Act engine)."""
    from concourse.hw_specs import get_activation_tables

    tabs = list(get_activation_tables(nc.m.arch).items())
    target = next(
        i
        for i, (_, funcs) in enumerate(tabs)
        if mybir.ActivationFunctionType.Exp in funcs
        and mybir.ActivationFunctionType.Ln in funcs
    )
    for blk in nc.main_func.blocks:
        loads = [
            ins
            for ins in blk.instructions
            if isinstance(ins, mybir.InstLoadActFuncSet)
        ]
        if not loads:
            continue
        loads[0].act_func_set_id = target
        for ins in loads[1:]:
            blk.instructions.remove(ins)


def _drop_dead_const_memsets(nc):
    """The framework materializes several [128,1] constant tiles (1.0 in two
    dtypes, uint8-127) with Pool memsets that serialize ahead of the entry
    barrier; only const-float32-0.0 (the activations' bias operand) is ever
    read here. Drop the unread ones -- each costs ~95 ns of pre-barrier
    Pool time."""
    read = set()
    for blk in nc.main_func.blocks:
        for ins in blk.instructions:
            try:
                for ap in list(ins.ins):
                    s = str(ap)
                    if "const-" in s:
                        read.add(s.split("memref='")[1].split("'")[0])
            except Exception:
                pass
    for blk in nc.main_func.blocks:
        dead = [
            ins
            for ins in blk.instructions
            if isinstance(ins, mybir.InstMemset)
            and "const-" in str(ins.outs[0])
            and str(ins.outs[0]).split("memref='")[1].split("'")[0] not in read
        ]
        for ins in dead:
            blk.instructions.remove(ins)
        # the surviving const memset (exp/ln bias tile) runs on DVE so the
        # preamble barrier isn't serialized behind Pool
        for ins in blk.instructions:
            if isinstance(ins, mybir.InstMemset) and "const-" in str(ins.outs[0]):
                ins.engine = mybir.EngineType.DVE


def _build_program() -> bass.Bass:
    if "nc" in _PROGRAM_CACHE:
        return _PROGRAM_CACHE["nc"]

    nc = bacc.Bacc("TRN2", target_bir_lowering=False, debug=False)
    xb0_d = nc.dram_tensor("xb0", [128, 534], bf16, kind="ExternalInput").ap()
    xb1_d = nc.dram_tensor("xb1", [128, 534], bf16, kind="ExternalInput").ap()
    am0_d = nc.dram_tensor("am0", [128, 300], bf16, kind="ExternalInput").ap()
    am1_d = nc.dram_tensor("am1", [128, 300], bf16, kind="ExternalInput").ap()
    wxa_d = nc.dram_tensor("wxa", [128, 556], bf16, kind="ExternalInput").ap()
    xtb_d = nc.dram_tensor("xtb", [128, 300], bf16, kind="ExternalInput").ap()
    op_d = nc.dram_tensor("op", [128, BPC * S], bf16, kind="ExternalOutput").ap()

    with tile.TileContext(nc) as tc:
        with (
            tc.tile_pool(name="const", bufs=1) as cpool,
            tc.tile_pool(name="work", bufs=1) as wpool,
            tc.tile_pool(name="psum", bufs=1, space="PSUM") as ppool,
        ):
            xb = [
                wpool.tile([128, 534], bf16, tag=f"xb{h}", name=f"xb{h}")
                for h in range(2)
            ]
            am = [
                wpool.tile([128, 300], bf16, tag=f"am{h}", name=f"am{h}")
                for h in range(2)
            ]
            wxa = cpool.tile([128, 556], bf16, tag="wxa")
            xtb = cpool.tile([128, 300], bf16, tag="xtb")

            # Input DMAs. Bus/sem landing order by design: xb0, xb1, am0,
            # am1, wxa, xtb -- matching the order consumers need them.
            nc.sync.dma_start(xb[0][:], xb0_d[:, :])
            nc.gpsimd.dma_start(xb[1][:], xb1_d[:, :])
            nc.scalar.dma_start(am[0][:], am0_d[:, :])
            nc.sync.dma_start(am[1][:], am1_d[:, :])
            nc.gpsimd.dma_start(wxa[:], wxa_d[:, :])
            nc.sync.dma_start(xtb[:], xtb_d[:, :])

            wst = wxa[:, 0:D]
            wnt = wxa[:, D : 2 * D]

            # Load the one activation table that serves BOTH exp and ln up
            # front. Without this, the greedy insertion pass (and, worse,
            # the tile scheduler's internal cost model) charges a 1283 ns
            # table load before the first exp AND before the first Ln --
            # the latter skews the scheduler into thinking exp finishes
            # late, which made it front-load the wxa/xtb-gated self matmuls
            # ahead of the M matmuls on the in-order PE stream.
            from concourse.hw_specs import get_activation_tables

            _tabs = list(get_activation_tables(nc.m.arch).items())
            _combined = next(
                i
                for i, (_, funcs) in enumerate(_tabs)
                if mybir.ActivationFunctionType.Exp in funcs
                and mybir.ActivationFunctionType.Ln in funcs
            )
            _atl = mybir.InstLoadActFuncSet(
                name=nc.get_next_instruction_name(),
                ins=[],
                outs=[],
                act_func_set_id=_combined,
            )
            nc.scalar.add_instruction(_atl)

            # E = exp(t*x), one op per half (mains + resident x-tails;
            # the raw A-tail cols 384:534 are not exp'd)
            ec = [
                wpool.tile([128, 384], bf16, tag=f"ec{h}", name=f"ec{h}")
                for h in range(2)
            ]
            for h in range(2):
                nc.scalar.activation(
                    ec[h][:],
                    xb[h][:, 0:384],
                    mybir.ActivationFunctionType.Exp,
                    scale=T_LSE,
                )

            # PE pstate warmup: the tensor engine reaches full speed only
            # after ~3us of continuous execution. Keep it busy from the
            # start with dummy K=1 matmuls over a memset scratch row so the
            # real matmuls issue into a warm (full-rate) engine.
            wsrc = wpool.tile([1, 520], bf16, tag="wsrc")
            wps = ppool.tile([1, 512], f32, tag="wps")
            nc.vector.memset(wsrc[:], 0.0)
            for w in range(5):
                nc.tensor.matmul(
                    wps[:], wsrc[0:1, 0:1], wsrc[0:1, 8:520], start=True, stop=True
                )

            mM = [ppool.tile([128, 2 * S], f32, tag=f"mM{p}", name=f"mM{p}") for p in range(2)]
            mO = [
                ppool.tile([128, S], f32, tag=f"mO{b}", name=f"mO{b}")[:]
                for b in range(BPC)
            ]

            # M'^T = sum_j E[j,d] * A[j,i] (+ ones-row), per batch; all M
            # matmuls emitted before any O work so PE never stalls on Ln.
            for b in range(BPC):
                p, q = divmod(b, 2)
                th, tr = TAIL_HOME[b]
                nc.tensor.matmul(
                    mM[p][:, q * S : (q + 1) * S],
                    ec[p][:, q * D : (q + 1) * D],
                    am[p][:, q * S : (q + 1) * S],
                    start=True,
                    stop=False,
                )
                nc.tensor.matmul(
                    mM[p][:, q * S : (q + 1) * S],
                    ec[th][tr : tr + TR, 256:384],
                    xb[th][tr : tr + TR, 384:534],
                    start=False,
                    stop=True,
                )

            nT = wpool.tile([128, BPC * S], bf16, tag="nT")
            osb = wpool.tile([128, BPC * S], bf16, tag="osb")

            # nT = ln(2^-64 * M') in bf16, straight from PSUM, per pair
            for p in range(2):
                nc.scalar.activation(
                    nT[:, p * 2 * S : (p + 1) * 2 * S],
                    mM[p][:],
                    mybir.ActivationFunctionType.Ln,
                    scale=2.0**-64,
                )

            # Output matmuls per batch: self terms first (ready early via
            # wxt), neighbor terms interleaved to close each batch's group
            # as soon as its nT half lands.
            # The self matmul OPENS each batch's PSUM group and the cheap
            # neighbor term closes it, so the output copies gate on the
            # neighbor matmul alone. To stop the scheduler hoisting the
            # self matmuls ahead of the M matmuls (in-order PE stream ->
            # head-of-line blocking), one element of each self rhs region is
            # "poked" in place by a value-preserving DVE op (0*ec1 + self)
            # that REALLY reads exp pair 1's output, so the selfs are
            # dependency-ordered after the exp chain.
            def _poke(ap_elem):
                nc.vector.scalar_tensor_tensor(
                    out=ap_elem,
                    in0=ec[1][0:1, 0:1],
                    scalar=0.0,
                    in1=ap_elem,
                    op0=mybir.AluOpType.mult,
                    op1=mybir.AluOpType.add,
                )

            _poke(wxa[0:1, 256:257])
            _poke(wxa[0:1, 256 + S : 257 + S])
            _poke(xtb[0:1, 0:1])
            _poke(xtb[0:1, S : S + 1])

            def self_mm(b):
                rhs = (
                    wxa[:, 256 + b * S : 256 + (b + 1) * S]
                    if b < 2
                    else xtb[:, (b - 2) * S : (b - 1) * S]
                )
                nc.tensor.matmul(mO[b], wst, rhs, start=True, stop=False)

            def neigh_mm(b):
                nc.tensor.matmul(
                    mO[b],
                    wnt,
                    nT[:, b * S : (b + 1) * S],
                    start=False,
                    stop=True,
                )

            for b in range(BPC):
                self_mm(b)
            for b in range(BPC):
                neigh_mm(b)

            # PSUM -> SBUF staging per batch, alternating DVE / Act
            for b in range(BPC):
                dst = osb[:, b * S : (b + 1) * S]
                if b in (0, 3):
                    nc.scalar.copy(dst, mO[b])
                else:
                    nc.vector.tensor_copy(out=dst, in_=mO[b])

            # two output DMAs; outA is emitted after every copy so the Act
            # SEQ (queue depth 0) never blocks a pending copy on its wait.
            nc.scalar.dma_start(op_d[:, 0 : 2 * S], osb[:, 0 : 2 * S])
            nc.sync.dma_start(op_d[:, 2 * S : 4 * S], osb[:, 2 * S : 4 * S])

    nc.compile()
    _merge_act_table_loads(nc)
    _drop_dead_const_memsets(nc)
    _PROGRAM_CACHE["nc"] = nc
    return nc


def pack_inputs(x, A, W_self, W_neigh):
    """Per-core input dicts; all packing/casting on host."""
    x = np.asarray(x, dtype=np.float32)
    A = np.asarray(A)
    wst = np.ascontiguousarray(np.asarray(W_self, dtype=np.float32).T).astype(
        ml_dtypes.bfloat16
    )
    wnt = np.ascontiguousarray(
        np.asarray(W_neigh, dtype=np.float32).T / np.float32(T_LSE)
    ).astype(ml_dtypes.bfloat16)

    maps = []
    for c in range(NCORES):
        xs = x[c * BPC : (c + 1) * BPC]  # [BPC, S, D]
        As = A[c * BPC : (c + 1) * BPC]  # [BPC, S, S]
        xba = [np.zeros((128, 534), dtype=ml_dtypes.bfloat16) for _ in range(2)]
        ama = [np.zeros((128, 300), dtype=ml_dtypes.bfloat16) for _ in range(2)]
        for b in range(BPC):
            p, q = divmod(b, 2)
            th, tr = TAIL_HOME[b]
            xcast = xs[b].astype(ml_dtypes.bfloat16)
            acast = As[b].astype(ml_dtypes.bfloat16)
            xba[p][:, q * D : (q + 1) * D] = xcast[:J0, :]
            xba[th][tr : tr + JT, 256:384] = xcast[J0:, :]
            xba[th][tr : tr + JT, 384:534] = acast[J0:, :]
            xba[th][tr + JT, 384:534] = np.float32(1.0)
            ama[p][:, q * S : (q + 1) * S] = acast[:J0, :]
        xT = (
            np.ascontiguousarray(xs.transpose(2, 0, 1))
            .reshape(D, BPC * S)
            .astype(ml_dtypes.bfloat16)
        )
        wxa = np.ascontiguousarray(
            np.concatenate([wst, wnt, xT[:, 0 : 2 * S]], axis=1)
        )
        xtb = np.ascontiguousarray(xT[:, 2 * S : 4 * S])
        maps.append(
            {
                "xb0": xba[0],
                "xb1": xba[1],
                "am0": ama[0],
                "am1": ama[1],
                "wxa": wxa,
                "xtb": xtb,
            }
        )
    return maps


def _bias(W_neigh):
    """Host-side correction for the -64*ln2 Ln shift through W_neigh."""
    return np.float32(LN2_64) * (
        np.asarray(W_neigh, dtype=np.float32) / np.float32(T_LSE)
    ).sum(axis=1)


def unpack_output(res_out, bias):
    """op [D, BPC*S] (= out^T, e-major) + bias[e] -> [BPC, S, D]"""
    o = np.asarray(res_out, dtype=np.float32) + bias[:, None]
    return np.ascontiguousarray(o.reshape(D, BPC, S).transpose(1, 2, 0))


def kernel(x, A, W_self, W_neigh, **kwargs):
    nc = _build_program()
    in_maps = pack_inputs(x, A, W_self, W_neigh)
    bias = _bias(W_neigh)
    res = run_bass_kernel_spmd(nc, in_maps, core_ids=list(range(NCORES)), **kwargs)
    out = np.concatenate(
        [unpack_output(res.results[c]["op"], bias) for c in range(NCORES)], axis=0
    )
    return np.ascontiguousarray(out.astype(np.float32))
